# revision 2
# baseline (speedup 1.0000x reference)
"""Trainium2 Bass kernel for nn_CustomAttn: fused QKV + RoPE + causal SDPA + out-proj.

Sharding: tensor-parallel over heads (16 heads / 8 cores = 2 heads/core).
Each core computes QKV for its 2 heads (d-major layouts), RoPE, causal
flash-style attention (scores kept transposed [k, q] so softmax-normalization
and the PV matmul need no per-block transposes), producing attn^T feature-major
[128, tokens]. An AllGather over the partition axis assembles the full
attn^T [1024, tokens]; each core then computes its 128-row slice of
y^T = w_out @ attn^T.  Host assembles y from the 8 row-slices.

All matmuls run in float32r (TF32-like: ~1.5e-4 rel err, 4x faster than fp32).
"""
import sys

if "/opt/trn_rl_repo" not in sys.path:
    sys.path.insert(0, "/opt/trn_rl_repo")

import numpy as np

import concourse.bass as bass
import concourse.tile as tile
from concourse import bacc, mybir
from concourse.bass_utils import run_bass_kernel_spmd
from concourse.masks import make_identity

F32 = mybir.dt.float32
F32R = mybir.dt.float32r
EXP = mybir.ActivationFunctionType.Exp

B, S, D, H, HD = 2, 2048, 1024, 16, 64
NCORE = 8
HPC = H // NCORE  # 2 heads per core
TOK = B * S  # 4096 flattened tokens
ST = 512  # s-tile / q-tile width
NST = TOK // ST  # 8
NQT = S // ST  # 4 q-tiles per batch
KB = 128  # k-block
NKB_B = S // KB  # 16 k-blocks per batch
DCH = D // 128  # 8 contraction chunks
SCALE = 1.0 / np.sqrt(HD)
ROPE_BASE = 10000.0

_CACHE: dict = {}


def _build_program(collective: bool = True):
    nc = bacc.Bacc("TRN2", target_bir_lowering=False, debug=False, num_devices=NCORE)

    # ---- DRAM I/O ----
    xT_d = nc.dram_tensor("xT", [D, TOK], F32R, kind="ExternalInput").ap()
    wq_d = nc.dram_tensor("wq", [D, 128], F32R, kind="ExternalInput").ap()
    wk_d = nc.dram_tensor("wk", [D, 128], F32R, kind="ExternalInput").ap()
    wv_d = nc.dram_tensor("wv", [D, 128], F32R, kind="ExternalInput").ap()
    wo_d = nc.dram_tensor("wo", [D, 128], F32R, kind="ExternalInput").ap()
    cos_d = nc.dram_tensor("cosT", [128, S], F32, kind="ExternalInput").ap()
    sin_d = nc.dram_tensor("sinT", [128, S], F32, kind="ExternalInput").ap()
    yt_d = nc.dram_tensor("yt", [128, TOK], F32, kind="ExternalOutput").ap()

    with tile.TileContext(nc) as tc:
        with (
            tc.tile_pool(name="const", bufs=1) as cpool,
            tc.tile_pool(name="persist", bufs=1) as ppool,
            tc.tile_pool(name="xt", bufs=2) as xpool,
            tc.tile_pool(name="rope", bufs=2) as rpool,
            tc.tile_pool(name="e", bufs=6) as epool,
            tc.tile_pool(name="at", bufs=2) as apool,
            tc.tile_pool(name="rz", bufs=2) as zpool,
            tc.tile_pool(name="agin", bufs=2) as gpool,
            tc.tile_pool(name="yt", bufs=2) as ypool,
            tc.tile_pool(name="pqkv", bufs=2, space="PSUM") as pqkv,
            tc.tile_pool(name="pscr", bufs=4, space="PSUM") as pscr,
            tc.tile_pool(name="po", bufs=2, space="PSUM") as po,
            tc.tile_pool(name="dram", bufs=1, space="DRAM") as dpool,
        ):
            # ---- constants / weights ----
            wq_sb = cpool.tile([128, DCH, 128], F32R)
            nc.sync.dma_start(wq_sb[:], wq_d.rearrange("(a p) m -> p a m", p=128))
            wk_sb = cpool.tile([128, DCH, 128], F32R)
            nc.sync.dma_start(wk_sb[:], wk_d.rearrange("(a p) m -> p a m", p=128))
            wv_sb = cpool.tile([128, DCH, 128], F32R)
            nc.sync.dma_start(wv_sb[:], wv_d.rearrange("(a p) m -> p a m", p=128))
            wo_sb = cpool.tile([128, DCH, 128], F32R)
            cos_sb = cpool.tile([128, S], F32)
            sin_sb = cpool.tile([128, S], F32)

            nc.gpsimd.dma_start(cos_sb[:], cos_d)
            nc.gpsimd.dma_start(sin_sb[:], sin_d)
            nc.gpsimd.dma_start(wo_sb[:], wo_d.rearrange("(a p) m -> p a m", p=128))
            id_sb = cpool.tile([128, 128], F32)
            make_identity(nc, id_sb[:])
            onesf = cpool.tile([128, 1], F32)
            nc.vector.memset(onesf[:], 1.0)

            # ---- persistent activations ----
            qt_all = ppool.tile([128, TOK], F32R)  # RoPE'd Q^T (2 heads stacked)
            kt_all = ppool.tile([128, TOK], F32R)  # RoPE'd K^T
            # token-major V per 128-token block, per-head [64 V | 1 ones] slots
            v_all = ppool.tile([128, 2 * NKB_B, 2 * (HD + 1)], F32R)

            def rope(dst, src_ps, s0):
                """dst[128,ST] (f32r) = src*cos + rotate_half(src)*sin_signed."""
                stg = rpool.tile([128, ST], F32, tag="stg")
                nc.vector.tensor_copy(stg[:], src_ps[:])
                rot = rpool.tile([128, ST], F32, tag="rot")
                for h0 in (0, 64):
                    nc.gpsimd.tensor_copy(
                        rot[h0 : h0 + 32, :], stg[h0 + 32 : h0 + 64, :]
                    )
                    nc.gpsimd.tensor_copy(
                        rot[h0 + 32 : h0 + 64, :], stg[h0 : h0 + 32, :]
                    )
                t1 = rpool.tile([128, ST], F32, tag="t1")
                nc.vector.tensor_mul(t1[:], stg[:], cos_sb[:, s0 : s0 + ST])
                nc.vector.tensor_mul(rot[:], rot[:], sin_sb[:, s0 : s0 + ST])
                nc.vector.tensor_add(dst, t1[:], rot[:])

            # ---- phases 1+2 interleaved: QKV tile then attention per (b, qt) ----
            ag_in = {}
            ag_out = {}
            for b in range(B):
                for qt in range(NQT):
                    ag_in[b, qt] = dpool.tile(
                        [128, ST], F32R, name=f"ag_in{b}_{qt}"
                    )
                    ag_out[b, qt] = dpool.tile(
                        [D, ST], F32R, addr_space="Shared", name=f"ag_out{b}_{qt}"
                    )

            def emit_qkv_tile(st):
                s0 = (st % NQT) * ST  # within-batch position (cos/sin index)
                tok0 = st * ST
                xt_sb = xpool.tile([128, DCH, ST], F32R, tag="xt", name=f"xt{st}")
                xr = xT_d.rearrange("(a p) m -> p a m", p=128)
                if st == 0:  # split so the first matmuls start sooner
                    nc.sync.dma_start(
                        xt_sb[:, 0:4, :], xr[:, 0:4, tok0 : tok0 + ST]
                    )
                    nc.sync.dma_start(
                        xt_sb[:, 4:DCH, :], xr[:, 4:DCH, tok0 : tok0 + ST]
                    )
                else:
                    nc.sync.dma_start(xt_sb[:], xr[:, :, tok0 : tok0 + ST])
                for part, w_sb in (("q", wq_sb), ("k", wk_sb), ("v", wv_sb)):
                    acc = pqkv.tile([128, ST], F32, tag="qkv", name=f"ps_{part}{st}")
                    for dk in range(DCH):
                        nc.tensor.matmul(
                            acc[:],
                            w_sb[:, dk, :],
                            xt_sb[:, dk, :],
                            start=(dk == 0),
                            stop=(dk == DCH - 1),
                        )
                    if part == "q":
                        rope(qt_all[:, tok0 : tok0 + ST], acc, s0)
                    elif part == "k":
                        rope(kt_all[:, tok0 : tok0 + ST], acc, s0)
                    else:
                        vs = rpool.tile([128, ST], F32, tag="stg", name=f"vs{st}")
                        nc.vector.tensor_copy(vs[:], acc[:])
                        slot = st * (ST // KB)
                        tr = pscr.tile([128, ST], F32, tag="scr", name=f"tr{st}")
                        for jj in range(ST // KB):
                            nc.tensor.transpose(
                                tr[:, jj * 128 : (jj + 1) * 128],
                                vs[:, jj * 128 : (jj + 1) * 128],
                                id_sb[:],
                            )
                        trv = tr[:].rearrange("p (j t h) -> p j t h", j=4, t=2)
                        for h in range(HPC):
                            c0 = h * (HD + 1)
                            nc.vector.tensor_copy(
                                v_all[:, slot : slot + 4, c0 : c0 + HD],
                                trv[:, :, h, :],
                            )
                            for jj in range(ST // KB):
                                nc.gpsimd.tensor_copy(
                                    v_all[:, slot + jj, c0 + HD : c0 + HD + 1],
                                    onesf[:],
                                )

            def emit_attention(b, qt):
                q0 = b * S + qt * ST
                nkb = (qt + 1) * (ST // KB)
                o_ps = [
                    po.tile([HD + 1, ST], F32, tag="o", name=f"o{b}_{qt}_{h}")
                    for h in range(HPC)
                ]
                e_tiles = {}

                def emit_scores(kbi):
                    k0 = b * S + kbi * KB
                    for h in range(HPC):
                        stp = pscr.tile(
                            [128, ST], F32, tag="scr", name=f"st{b}_{qt}_{kbi}_{h}"
                        )
                        nc.tensor.matmul(
                            stp[:],
                            kt_all[h * HD : (h + 1) * HD, k0 : k0 + KB],
                            qt_all[h * HD : (h + 1) * HD, q0 : q0 + ST],
                            start=True,
                            stop=True,
                        )
                        e_sb = epool.tile(
                            [128, ST], F32R, tag="e", name=f"e{b}_{qt}_{kbi}_{h}"
                        )
                        nc.scalar.activation(e_sb[:], stp[:], EXP, scale=SCALE)
                        j = kbi - qt * (ST // KB)
                        if j >= 0:
                            # causal: keep where q - k - j*128 >= 0
                            nc.gpsimd.affine_select(
                                out=e_sb[:],
                                in_=e_sb[:],
                                compare_op=mybir.AluOpType.is_ge,
                                fill=0.0,
                                base=-j * KB,
                                pattern=[[1, ST]],
                                channel_multiplier=-1,
                            )
                        e_tiles[kbi, h] = e_sb

                def emit_pv(kbi):
                    slot = b * NKB_B + kbi
                    for h in range(HPC):
                        c0 = h * (HD + 1)
                        nc.tensor.matmul(
                            o_ps[h][:],
                            v_all[:, slot, c0 : c0 + HD + 1],
                            e_tiles.pop((kbi, h))[:],
                            start=(kbi == 0),
                            stop=(kbi == nkb - 1),
                        )

                emit_scores(0)
                for kbi in range(1, nkb):
                    emit_scores(kbi)
                    emit_pv(kbi - 1)
                emit_pv(nkb - 1)
                o2_sb = zpool.tile([128, ST], F32, tag="osb")
                rz = zpool.tile([64, ST], F32, tag="rz")
                for h in range(HPC):
                    nc.any.tensor_copy(
                        o2_sb[h * HD : (h + 1) * HD, :], o_ps[h][0:HD, :]
                    )
                    nc.vector.reciprocal(
                        rz[h * 32 : h * 32 + 1, :], o_ps[h][HD : HD + 1, :]
                    )
                zb = dpool.tile([HPC, ST], F32, tag="rzb", bufs=4, name=f"zb{b}_{qt}")
                nc.sync.dma_start(
                    zb[:], rz[:].rearrange("(a p) m -> a p m", p=32)[:, 0, :]
                )
                bc_sb = zpool.tile([128, ST], F32, tag="bcs")
                nc.sync.dma_start(
                    bc_sb[:],
                    zb[:].rearrange("h (o m) -> h o m", o=1).to_broadcast([HPC, HD, ST]),
                )
                at_sb = apool.tile([128, ST], F32R, tag="at")
                nc.vector.tensor_mul(at_sb[:], o2_sb[:], bc_sb[:])
                nc.sync.dma_start(ag_in[b, qt][:], at_sb[:])

            def emit_ag(b, qt):
                if collective:
                    nc.gpsimd.collective_compute(
                        "AllGather",
                        mybir.AluOpType.bypass,
                        replica_groups=[list(range(NCORE))],
                        ins=[ag_in[b, qt].opt()],
                        outs=[ag_out[b, qt].opt()],
                    )
                else:  # timing-only single-core stand-in (replicate to all slices)
                    nc.sync.dma_start(
                        ag_out[b, qt][:],
                        ag_in[b, qt][:]
                        .rearrange("p (o m) -> o p m", o=1)
                        .to_broadcast([NCORE, 128, ST]),
                    )

            def emit_outproj(b, qt, split=False):
                tt = b * NQT + qt
                ag_sb = gpool.tile([128, DCH, ST], F32R, tag="ag", name=f"ag{tt}")
                agr = ag_out[b, qt].rearrange("(a p) m -> p a m", p=128)
                if split:
                    for fk in range(DCH):
                        nc.sync.dma_start(ag_sb[:, fk, :], agr[:, fk, :])
                else:
                    nc.sync.dma_start(ag_sb[:], agr[:])
                yt_ps = pqkv.tile([128, ST], F32, tag="qkv", name=f"yt{tt}")
                for fk in range(DCH):
                    nc.tensor.matmul(
                        yt_ps[:],
                        wo_sb[:, fk, :],
                        ag_sb[:, fk, :],
                        start=(fk == 0),
                        stop=(fk == DCH - 1),
                    )
                yt_sb = ypool.tile([128, ST], F32, tag="yt")
                nc.vector.tensor_copy(yt_sb[:], yt_ps[:])
                nc.sync.dma_start(yt_d[:, tt * ST : (tt + 1) * ST], yt_sb[:])

            emit_qkv_tile(0)
            for st in range(NST):
                b, qt = st // NQT, st % NQT
                if st + 1 < NST:
                    emit_qkv_tile(st + 1)
                emit_attention(b, qt)
                emit_ag(b, qt)
                if st >= 1:
                    emit_outproj((st - 1) // NQT, (st - 1) % NQT)
            emit_outproj(B - 1, NQT - 1, split=True)

    nc.compile()
    return nc


def _host_tables():
    inv_freq = 1.0 / (ROPE_BASE ** (np.arange(0, HD, 2, dtype=np.float32) / HD))
    t = np.arange(S, dtype=np.float32)
    freqs = np.outer(t, inv_freq)  # [S, 32]
    emb = np.concatenate([freqs, freqs], axis=-1)  # [S, 64]
    cos = np.cos(emb).astype(np.float32)
    sin = np.sin(emb).astype(np.float32)
    sinS = np.concatenate([-sin[:, : HD // 2], sin[:, HD // 2 :]], axis=1)
    cosT2 = np.ascontiguousarray(np.concatenate([cos.T, cos.T], axis=0))  # [128,S]
    sinT2 = np.ascontiguousarray(np.concatenate([sinS.T, sinS.T], axis=0))
    return cosT2, sinT2


def _get_nc():
    if "nc" not in _CACHE:
        _CACHE["nc"] = _build_program()
        _CACHE["tables"] = _host_tables()
    return _CACHE["nc"]


def _make_in_maps(x, w_in, w_out):
    cosT2, sinT2 = _CACHE["tables"]
    xT = np.ascontiguousarray(x.reshape(TOK, D).T)  # [D, TOK]
    in_maps = []
    for c in range(NCORE):
        r = slice(c * 128, (c + 1) * 128)
        in_maps.append(
            {
                "xT": xT,
                "wq": np.ascontiguousarray(w_in[0 * D :][r.start : r.stop].T),
                "wk": np.ascontiguousarray(w_in[1 * D :][r.start : r.stop].T),
                "wv": np.ascontiguousarray(w_in[2 * D :][r.start : r.stop].T),
                "wo": np.ascontiguousarray(w_out[r, :].T),
                "cosT": cosT2,
                "sinT": sinT2,
            }
        )
    return in_maps


def kernel(x: np.ndarray, w_in: np.ndarray, w_out: np.ndarray) -> np.ndarray:
    x = np.asarray(x, dtype=np.float32)
    w_in = np.asarray(w_in, dtype=np.float32)
    w_out = np.asarray(w_out, dtype=np.float32)

    nc = _get_nc()
    in_maps = _make_in_maps(x, w_in, w_out)
    _CACHE["last_in_maps"] = in_maps
    res = run_bass_kernel_spmd(nc, in_maps, core_ids=list(range(NCORE)))
    yT = np.concatenate([res.results[c]["yt"] for c in range(NCORE)], axis=0)
    return np.ascontiguousarray(yT.T).reshape(B, S, D)



# revision 16
# speedup vs baseline: 1.4314x; 1.4314x over previous
"""Trainium2 Bass kernel for nn_CustomAttn: fused QKV + RoPE + causal SDPA + out-proj.

Sharding: tensor-parallel over heads (16 heads / 8 cores = 2 heads/core).
Each core computes QKV for its 2 heads (d-major layouts), RoPE, causal
flash-style attention (scores kept transposed [k, q]), producing normalized
attn^T feature-major [128, 512] per 512-token tile, cast to bf16.  A per-tile
AllGather (bf16 payload: half the wire bytes / readback of fp32) assembles the
full attn^T [1024, 512]; each core then computes its 128-row slice of
y^T = w_out @ attn^T.  Host assembles y from the 8 row-slices.
(AllToAll would cut collective traffic 8x further, but it is broken on this
runtime: returns nondeterministic garbage. Verified by micro-test.)

vs the v1 baseline:
 - bf16 AllGather payload + bf16 out-proj weights (wire 14 MB -> 7 MB,
   readback 16 MB -> 8 MB per core).
 - readback + out-proj for tile t deferred until after the AG trigger of
   tile t+2, so the sync-queue DMA never stalls behind an in-flight
   collective (v1 blocked xt loads there, starving the tensor engine).
 - rotate_half runs as a signed-permutation matmul on TensorE (was two
   GpSimd partition-copies per rope, ~130us of GpSimd time).
 - softmax 1/z runs on ScalarE as Exp(-Ln(z)) (same act table set as the
   main Exp, no table reloads; was vector.reciprocal, ~3.3us per call).
 - 1/z broadcast across partitions via a tiny selection matmul (was a DMA
   round-trip through DRAM).
 - Q kept per-tile instead of a persistent [128, 4096] buffer.

Matmuls run in float32r (TF32-like, full PE rate at free-dim >= 256); the
out-projection runs in bf16.
"""
import sys

if "/opt/trn_rl_repo" not in sys.path:
    sys.path.insert(0, "/opt/trn_rl_repo")

import numpy as np

import concourse.bass as bass
import concourse.tile as tile
from concourse import bacc, mybir
from concourse.bass_utils import run_bass_kernel_spmd
from concourse.masks import make_identity

F32 = mybir.dt.float32
F32R = mybir.dt.float32r
BF16 = mybir.dt.bfloat16
EXP = mybir.ActivationFunctionType.Exp
LN = mybir.ActivationFunctionType.Ln

B, S, D, H, HD = 2, 2048, 1024, 16, 64
NCORE = 8
HPC = H // NCORE  # 2 heads per core
TOK = B * S  # 4096 flattened tokens
ST = 512  # s-tile / q-tile width
NST = TOK // ST  # 8
NQT = S // ST  # 4 q-tiles per batch
KB = 128  # k-block
NKB_B = S // KB  # 16 k-blocks per batch
DCH = D // 128  # 8 contraction chunks
TPC = ST // NCORE  # 64 tokens per core per tile (a2a shard)
SCALE = 1.0 / np.sqrt(HD)
ROPE_BASE = 10000.0

_CACHE: dict = {}


def _build_program():
    nc = bacc.Bacc("TRN2", target_bir_lowering=False, debug=False, num_devices=NCORE)

    # ---- DRAM I/O ----
    xT_d = nc.dram_tensor("xT", [D, TOK], F32R, kind="ExternalInput").ap()
    wq_d = nc.dram_tensor("wq", [D, 128], F32R, kind="ExternalInput").ap()
    wk_d = nc.dram_tensor("wk", [D, 128], F32R, kind="ExternalInput").ap()
    wv_d = nc.dram_tensor("wv", [D, 128], F32R, kind="ExternalInput").ap()
    wo_d = nc.dram_tensor("wo", [D, 128], BF16, kind="ExternalInput").ap()
    cos_d = nc.dram_tensor("cosT", [128, S], F32, kind="ExternalInput").ap()
    sin_d = nc.dram_tensor("sinT", [128, S], F32, kind="ExternalInput").ap()
    prot_d = nc.dram_tensor("prot", [128, 128], F32R, kind="ExternalInput").ap()
    zsel_d = nc.dram_tensor("zsel", [64, 128], F32R, kind="ExternalInput").ap()
    yt_d = nc.dram_tensor("yt", [128, TOK], F32, kind="ExternalOutput").ap()

    with tile.TileContext(nc) as tc:
        with (
            tc.tile_pool(name="const", bufs=1) as cpool,
            tc.tile_pool(name="persist", bufs=1) as ppool,
            tc.tile_pool(name="xt", bufs=2) as xpool,
            tc.tile_pool(name="rope", bufs=2) as rpool,
            tc.tile_pool(name="e", bufs=6) as epool,
            tc.tile_pool(name="at", bufs=2) as apool,
            tc.tile_pool(name="agin", bufs=2) as gpool,
            tc.tile_pool(name="rz", bufs=2) as zpool,
            tc.tile_pool(name="yt", bufs=2) as ypool,
            tc.tile_pool(name="ps", bufs=1, space="PSUM") as pspool,
            tc.tile_pool(name="dram", bufs=1, space="DRAM") as dpool,
        ):
            # ---- constants / weights ----
            wq_sb = cpool.tile([128, DCH, 128], F32R)
            nc.sync.dma_start(wq_sb[:], wq_d.rearrange("(a p) m -> p a m", p=128))
            wk_sb = cpool.tile([128, DCH, 128], F32R)
            nc.sync.dma_start(wk_sb[:], wk_d.rearrange("(a p) m -> p a m", p=128))
            wv_sb = cpool.tile([128, DCH, 128], F32R)
            nc.sync.dma_start(wv_sb[:], wv_d.rearrange("(a p) m -> p a m", p=128))
            cos_sb = cpool.tile([128, S], F32)
            sin_sb = cpool.tile([128, S], F32)
            prot_sb = cpool.tile([128, 128], F32R)
            zsel_sb = cpool.tile([64, 128], F32R)
            wo_sb = cpool.tile([128, DCH, 128], BF16)

            nc.gpsimd.dma_start(cos_sb[:], cos_d)
            nc.gpsimd.dma_start(sin_sb[:], sin_d)
            nc.gpsimd.dma_start(prot_sb[:], prot_d)
            nc.gpsimd.dma_start(zsel_sb[:], zsel_d)
            nc.gpsimd.dma_start(wo_sb[:], wo_d.rearrange("(a p) m -> p a m", p=128))
            id_sb = cpool.tile([128, 128], F32)
            make_identity(nc, id_sb[:])
            onesf = cpool.tile([128, 1], F32)
            nc.vector.memset(onesf[:], 1.0)

            # ---- persistent activations ----
            kt_all = ppool.tile([128, TOK], F32R)  # RoPE'd K^T (2 heads stacked)
            # token-major V per 128-token block, per-head [64 V | 1 ones] slots
            v_all = ppool.tile([128, 2 * NKB_B, HPC * (HD + 1)], F32R)

            def rope(dst, src_ps, s0):
                """dst[128,ST] = src*cos + (P_signed @ src)*sin."""
                stg = rpool.tile([128, ST], F32R, tag="stg")
                nc.vector.tensor_copy(stg[:], src_ps[:])
                rot = pspool.tile([128, ST], F32, tag="scr", bufs=3, name="rot")
                nc.tensor.matmul(rot[:], prot_sb[:], stg[:], start=True, stop=True)
                t1 = rpool.tile([128, ST], F32, tag="t1")
                nc.vector.tensor_mul(t1[:], stg[:], cos_sb[:, s0 : s0 + ST])
                t2 = rpool.tile([128, ST], F32, tag="t2")
                nc.vector.tensor_mul(t2[:], rot[:], sin_sb[:, s0 : s0 + ST])
                nc.vector.tensor_add(dst, t1[:], t2[:])

            # ---- per-tile DRAM staging for the AllGather (bf16 payload) ----
            ag_in = {}
            ag_out = {}
            for st in range(NST):
                ag_in[st] = dpool.tile([128, ST], BF16, name=f"ag_in{st}")
                ag_out[st] = dpool.tile(
                    [D, ST], BF16, addr_space="Shared", name=f"ag_out{st}"
                )

            q_tiles = {}

            def emit_qkv_tile(st):
                s0 = (st % NQT) * ST  # within-batch position (cos/sin index)
                tok0 = st * ST
                xt_sb = xpool.tile([128, DCH, ST], F32R, tag="xt", name=f"xt{st}")
                xr = xT_d.rearrange("(a p) m -> p a m", p=128)
                if st == 0:  # split so the first matmuls start sooner
                    nc.sync.dma_start(xt_sb[:, 0:4, :], xr[:, 0:4, tok0 : tok0 + ST])
                    nc.sync.dma_start(
                        xt_sb[:, 4:DCH, :], xr[:, 4:DCH, tok0 : tok0 + ST]
                    )
                else:
                    nc.sync.dma_start(xt_sb[:], xr[:, :, tok0 : tok0 + ST])
                qt_sb = rpool.tile([128, ST], F32R, tag="qt", name=f"qt{st}")
                q_tiles[st] = qt_sb
                for part, w_sb in (("q", wq_sb), ("k", wk_sb), ("v", wv_sb)):
                    acc = pspool.tile(
                        [128, ST], F32, tag="qkv", bufs=2, name=f"ps_{part}{st}"
                    )
                    for dk in range(DCH):
                        nc.tensor.matmul(
                            acc[:],
                            w_sb[:, dk, :],
                            xt_sb[:, dk, :],
                            start=(dk == 0),
                            stop=(dk == DCH - 1),
                        )
                    if part == "q":
                        rope(qt_sb[:], acc, s0)
                    elif part == "k":
                        rope(kt_all[:, tok0 : tok0 + ST], acc, s0)
                    else:
                        vs = rpool.tile([128, ST], F32, tag="stg", name=f"vs{st}")
                        nc.vector.tensor_copy(vs[:], acc[:])
                        slot = st * (ST // KB)
                        tr = pspool.tile(
                            [128, ST], F32, tag="scr", bufs=3, name=f"tr{st}"
                        )
                        for jj in range(ST // KB):
                            nc.tensor.transpose(
                                tr[:, jj * 128 : (jj + 1) * 128],
                                vs[:, jj * 128 : (jj + 1) * 128],
                                id_sb[:],
                            )
                        trv = tr[:].rearrange("p (j t h) -> p j t h", j=4, t=HPC)
                        for h in range(HPC):
                            c0 = h * (HD + 1)
                            nc.vector.tensor_copy(
                                v_all[:, slot : slot + 4, c0 : c0 + HD],
                                trv[:, :, h, :],
                            )
                            for jj in range(ST // KB):
                                nc.gpsimd.tensor_copy(
                                    v_all[:, slot + jj, c0 + HD : c0 + HD + 1],
                                    onesf[:],
                                )

            def emit_attention(b, qt):
                st = b * NQT + qt
                q0 = qt * ST  # within this core's qt_sb (always full tile)
                nkb = (qt + 1) * (ST // KB)
                qt_sb = q_tiles.pop(st)
                o_ps = [
                    pspool.tile(
                        [HD + 1, ST], F32, tag="o", bufs=2, name=f"o{b}_{qt}_{h}"
                    )
                    for h in range(HPC)
                ]
                e_tiles = {}

                def emit_scores(kbi):
                    k0 = b * S + kbi * KB
                    for h in range(HPC):
                        stp = pspool.tile(
                            [128, ST],
                            F32,
                            tag="scr",
                            bufs=3,
                            name=f"st{b}_{qt}_{kbi}_{h}",
                        )
                        nc.tensor.matmul(
                            stp[:],
                            kt_all[h * HD : (h + 1) * HD, k0 : k0 + KB],
                            qt_sb[h * HD : (h + 1) * HD, :],
                            start=True,
                            stop=True,
                        )
                        e_sb = epool.tile(
                            [128, ST], F32R, tag="e", name=f"e{b}_{qt}_{kbi}_{h}"
                        )
                        nc.scalar.activation(e_sb[:], stp[:], EXP, scale=SCALE)
                        j = kbi - qt * (ST // KB)
                        if j >= 0:
                            # causal: keep where q - k - j*128 >= 0
                            nc.gpsimd.affine_select(
                                out=e_sb[:],
                                in_=e_sb[:],
                                compare_op=mybir.AluOpType.is_ge,
                                fill=0.0,
                                base=-j * KB,
                                pattern=[[1, ST]],
                                channel_multiplier=-1,
                            )
                        e_tiles[kbi, h] = e_sb

                def emit_pv(kbi):
                    slot = b * NKB_B + kbi
                    for h in range(HPC):
                        c0 = h * (HD + 1)
                        nc.tensor.matmul(
                            o_ps[h][:],
                            v_all[:, slot, c0 : c0 + HD + 1],
                            e_tiles.pop((kbi, h))[:],
                            start=(kbi == 0),
                            stop=(kbi == nkb - 1),
                        )

                emit_scores(0)
                for kbi in range(1, nkb):
                    emit_scores(kbi)
                    emit_pv(kbi - 1)
                emit_pv(nkb - 1)

                # normalize: at = o2 * broadcast(1/z); 1/z = exp(-ln z) on ScalarE
                o2_sb = zpool.tile([128, ST], F32, tag="osb")
                zbc = pspool.tile([128, ST], F32, tag="scr", bufs=3, name=f"zbc{st}")
                nrcp = zpool.tile([64, ST], F32R, tag="nrcp", name=f"nrcp{st}")
                for h in range(HPC):
                    nc.any.tensor_copy(
                        o2_sb[h * HD : (h + 1) * HD, :], o_ps[h][0:HD, :]
                    )
                    lnz = zpool.tile([1, ST], F32, tag="lnz", name=f"lnz{st}_{h}")
                    nc.scalar.activation(lnz[:], o_ps[h][HD : HD + 1, :], LN)
                    # 1/z row parked at partition h*32 so it base-matches zsel
                    nc.scalar.activation(
                        nrcp[h * 32 : h * 32 + 1, :], lnz[:], EXP, scale=-1.0
                    )
                    # accumulate per-head broadcast rows into one [128, ST] tile
                    nc.tensor.matmul(
                        zbc[:],
                        zsel_sb[h * 32 : h * 32 + 1, :],
                        nrcp[h * 32 : h * 32 + 1, :],
                        start=(h == 0),
                        stop=(h == HPC - 1),
                    )
                at_sb = apool.tile([128, ST], BF16, tag="at")
                nc.vector.tensor_mul(at_sb[:], o2_sb[:], zbc[:])
                nc.sync.dma_start(ag_in[st][:], at_sb[:])

            def emit_ag(st):
                nc.gpsimd.collective_compute(
                    "AllGather",
                    mybir.AluOpType.bypass,
                    replica_groups=[list(range(NCORE))],
                    ins=[ag_in[st].opt()],
                    outs=[ag_out[st].opt()],
                )

            def emit_outproj(st):
                # deferred 2 tiles after the AG trigger so this sync-queue DMA
                # never blocks later xt loads behind an in-flight collective
                ag_sb = gpool.tile([128, DCH, ST], BF16, tag="ag", name=f"ag{st}")
                nc.sync.dma_start(
                    ag_sb[:], ag_out[st].rearrange("(a p) m -> p a m", p=128)
                )
                op_ps = pspool.tile([128, ST], F32, tag="qkv", bufs=2, name=f"op{st}")
                for fk in range(DCH):
                    nc.tensor.matmul(
                        op_ps[:],
                        wo_sb[:, fk, :],
                        ag_sb[:, fk, :],
                        start=(fk == 0),
                        stop=(fk == DCH - 1),
                    )
                yt_sb = ypool.tile([128, ST], F32, tag="yt")
                nc.vector.tensor_copy(yt_sb[:], op_ps[:])
                nc.sync.dma_start(yt_d[:, st * ST : (st + 1) * ST], yt_sb[:])

            emit_qkv_tile(0)
            for st in range(NST):
                b, qt = st // NQT, st % NQT
                if st + 1 < NST:
                    emit_qkv_tile(st + 1)
                emit_attention(b, qt)
                emit_ag(st)
                if st >= 2:
                    emit_outproj(st - 2)
            emit_outproj(NST - 2)
            emit_outproj(NST - 1)

    nc.compile()
    return nc


def _host_tables():
    inv_freq = 1.0 / (ROPE_BASE ** (np.arange(0, HD, 2, dtype=np.float32) / HD))
    t = np.arange(S, dtype=np.float32)
    freqs = np.outer(t, inv_freq)  # [S, 32]
    emb = np.concatenate([freqs, freqs], axis=-1)  # [S, 64]
    cos = np.cos(emb).astype(np.float32)
    sin = np.sin(emb).astype(np.float32)
    cosT2 = np.ascontiguousarray(np.concatenate([cos.T, cos.T], axis=0))  # [128,S]
    sinT2 = np.ascontiguousarray(np.concatenate([sin.T, sin.T], axis=0))
    # signed rotate-half permutation, stacked per 64-row head block, transposed
    # for use as a matmul stationary (rot = prot^T @ x).
    R = np.zeros((128, 128), dtype=np.float32)
    for h0 in (0, 64):
        for r in range(32):
            R[h0 + r, h0 + 32 + r] = -1.0
            R[h0 + 32 + r, h0 + r] = 1.0
    prot = np.ascontiguousarray(R.T)
    # z-broadcast selector: row h*32 -> partitions h*64..(h+1)*64 (rows at
    # 32-multiples because engine partition bases must be multiples of 32)
    zsel = np.zeros((64, 128), dtype=np.float32)
    for h in range(HPC):
        zsel[h * 32, h * HD : (h + 1) * HD] = 1.0
    return cosT2, sinT2, prot, zsel


def _get_nc():
    if "nc" not in _CACHE:
        _CACHE["nc"] = _build_program()
        _CACHE["tables"] = _host_tables()
    return _CACHE["nc"]


def _make_in_maps(x, w_in, w_out):
    cosT2, sinT2, prot, zsel = _CACHE["tables"]
    import ml_dtypes

    xT = np.ascontiguousarray(x.reshape(TOK, D).T)  # [D, TOK]
    in_maps = []
    for c in range(NCORE):
        r = slice(c * 128, (c + 1) * 128)
        in_maps.append(
            {
                "xT": xT,
                "wq": np.ascontiguousarray(w_in[0 * D :][r.start : r.stop].T),
                "wk": np.ascontiguousarray(w_in[1 * D :][r.start : r.stop].T),
                "wv": np.ascontiguousarray(w_in[2 * D :][r.start : r.stop].T),
                "wo": np.ascontiguousarray(
                    w_out[r, :].T.astype(ml_dtypes.bfloat16)
                ),
                "cosT": cosT2,
                "sinT": sinT2,
                "prot": prot,
                "zsel": zsel,
            }
        )
    return in_maps


def kernel(x: np.ndarray, w_in: np.ndarray, w_out: np.ndarray) -> np.ndarray:
    x = np.asarray(x, dtype=np.float32)
    w_in = np.asarray(w_in, dtype=np.float32)
    w_out = np.asarray(w_out, dtype=np.float32)

    nc = _get_nc()
    in_maps = _make_in_maps(x, w_in, w_out)
    _CACHE["last_in_maps"] = in_maps
    res = run_bass_kernel_spmd(nc, in_maps, core_ids=list(range(NCORE)))
    yT = np.concatenate([res.results[c]["yt"] for c in range(NCORE)], axis=0)
    return np.ascontiguousarray(yT.T).reshape(B, S, D)


# revision 17
# speedup vs baseline: 1.5046x; 1.0511x over previous
"""Trainium2 Bass kernel for nn_CustomAttn: fused QKV + RoPE + causal SDPA + out-proj.

Sharding: tensor-parallel over heads (16 heads / 8 cores = 2 heads/core).
Each core computes QKV for its 2 heads (d-major layouts), RoPE, causal
flash-style attention (scores kept transposed [k, q]), producing normalized
attn^T feature-major [128, 512] per 512-token tile, cast to bf16.  A per-tile
AllGather (bf16 payload: half the wire bytes / readback of fp32) assembles the
full attn^T [1024, 512]; each core then computes its 128-row slice of
y^T = w_out @ attn^T.  Host assembles y from the 8 row-slices.
(AllToAll would cut collective traffic 8x further, but it is broken on this
runtime: returns nondeterministic garbage. Verified by micro-test.)

vs the v1 baseline:
 - bf16 AllGather payload + bf16 out-proj weights (wire 14 MB -> 7 MB,
   readback 16 MB -> 8 MB per core).
 - readback + out-proj for tile t deferred until after the AG trigger of
   tile t+2, so the sync-queue DMA never stalls behind an in-flight
   collective (v1 blocked xt loads there, starving the tensor engine).
 - rotate_half runs as a signed-permutation matmul on TensorE (was two
   GpSimd partition-copies per rope, ~130us of GpSimd time).
 - softmax 1/z runs on ScalarE as Exp(-Ln(z)) (same act table set as the
   main Exp, no table reloads; was vector.reciprocal, ~3.3us per call).
 - 1/z broadcast across partitions via a tiny selection matmul (was a DMA
   round-trip through DRAM).
 - Q kept per-tile instead of a persistent [128, 4096] buffer.

Matmuls run in float32r (TF32-like, full PE rate at free-dim >= 256); the
out-projection runs in bf16.
"""
import sys

if "/opt/trn_rl_repo" not in sys.path:
    sys.path.insert(0, "/opt/trn_rl_repo")

import numpy as np

import concourse.bass as bass
import concourse.tile as tile
from concourse import bacc, mybir
from concourse.bass_utils import run_bass_kernel_spmd
from concourse.masks import make_identity

# Keep Exp and Ln resolving to the ONE table set that holds both —
# otherwise the compiler alternates exp_and_others / natural_log and
# reloads ACT tables (~1.3us + pipeline flush) twice per tile.
import concourse.hw_specs as _hw_specs
import concourse.bacc as _bacc_mod

_orig_gat = _hw_specs.get_activation_tables


def _gat_one_exp_ln_set(arch):
    tabs = _orig_gat(arch)
    if "natural_log_exp_and_others" in tabs:
        for name, funcs in tabs.items():
            if name != "natural_log_exp_and_others":
                funcs.discard(mybir.ActivationFunctionType.Exp)
                funcs.discard(mybir.ActivationFunctionType.Ln)
    return tabs


_hw_specs.get_activation_tables = _gat_one_exp_ln_set
_bacc_mod.get_activation_tables = _gat_one_exp_ln_set

F32 = mybir.dt.float32
F32R = mybir.dt.float32r
BF16 = mybir.dt.bfloat16
EXP = mybir.ActivationFunctionType.Exp
LN = mybir.ActivationFunctionType.Ln

B, S, D, H, HD = 2, 2048, 1024, 16, 64
NCORE = 8
HPC = H // NCORE  # 2 heads per core
TOK = B * S  # 4096 flattened tokens
ST = 512  # s-tile / q-tile width
NST = TOK // ST  # 8
NQT = S // ST  # 4 q-tiles per batch
KB = 128  # k-block
NKB_B = S // KB  # 16 k-blocks per batch
DCH = D // 128  # 8 contraction chunks
TPC = ST // NCORE  # 64 tokens per core per tile (a2a shard)
SCALE = 1.0 / np.sqrt(HD)
ROPE_BASE = 10000.0

_CACHE: dict = {}


def _build_program():
    nc = bacc.Bacc("TRN2", target_bir_lowering=False, debug=False, num_devices=NCORE)

    # ---- DRAM I/O ----
    xT_d = nc.dram_tensor("xT", [D, TOK], F32R, kind="ExternalInput").ap()
    wq_d = nc.dram_tensor("wq", [D, 128], F32R, kind="ExternalInput").ap()
    wk_d = nc.dram_tensor("wk", [D, 128], F32R, kind="ExternalInput").ap()
    wv_d = nc.dram_tensor("wv", [D, 128], F32R, kind="ExternalInput").ap()
    wo_d = nc.dram_tensor("wo", [D, 128], BF16, kind="ExternalInput").ap()
    cos_d = nc.dram_tensor("cosT", [128, S], F32, kind="ExternalInput").ap()
    sin_d = nc.dram_tensor("sinT", [128, S], F32, kind="ExternalInput").ap()
    prot_d = nc.dram_tensor("prot", [128, 128], F32R, kind="ExternalInput").ap()
    zsel_d = nc.dram_tensor("zsel", [64, 128], F32R, kind="ExternalInput").ap()
    yt_d = nc.dram_tensor("yt", [128, TOK], F32, kind="ExternalOutput").ap()

    with tile.TileContext(nc) as tc:
        with (
            tc.tile_pool(name="const", bufs=1) as cpool,
            tc.tile_pool(name="persist", bufs=1) as ppool,
            tc.tile_pool(name="xt", bufs=2) as xpool,
            tc.tile_pool(name="rope", bufs=2) as rpool,
            tc.tile_pool(name="e", bufs=6) as epool,
            tc.tile_pool(name="at", bufs=2) as apool,
            tc.tile_pool(name="agin", bufs=2) as gpool,
            tc.tile_pool(name="rz", bufs=2) as zpool,
            tc.tile_pool(name="yt", bufs=2) as ypool,
            tc.tile_pool(name="ps", bufs=1, space="PSUM") as pspool,
            tc.tile_pool(name="dram", bufs=1, space="DRAM") as dpool,
        ):
            # ---- constants / weights ----
            wq_sb = cpool.tile([128, DCH, 128], F32R)
            nc.sync.dma_start(wq_sb[:], wq_d.rearrange("(a p) m -> p a m", p=128))
            wk_sb = cpool.tile([128, DCH, 128], F32R)
            nc.sync.dma_start(wk_sb[:], wk_d.rearrange("(a p) m -> p a m", p=128))
            wv_sb = cpool.tile([128, DCH, 128], F32R)
            nc.sync.dma_start(wv_sb[:], wv_d.rearrange("(a p) m -> p a m", p=128))
            cos_sb = cpool.tile([128, S], F32)
            sin_sb = cpool.tile([128, S], F32)
            prot_sb = cpool.tile([128, 128], F32R)
            zsel_sb = cpool.tile([64, 128], F32R)
            wo_sb = cpool.tile([128, DCH, 128], BF16)

            nc.gpsimd.dma_start(cos_sb[:], cos_d)
            nc.gpsimd.dma_start(sin_sb[:], sin_d)
            nc.gpsimd.dma_start(prot_sb[:], prot_d)
            nc.gpsimd.dma_start(zsel_sb[:], zsel_d)
            nc.gpsimd.dma_start(wo_sb[:], wo_d.rearrange("(a p) m -> p a m", p=128))
            id_sb = cpool.tile([128, 128], F32)
            make_identity(nc, id_sb[:])
            onesf = cpool.tile([128, 1], F32)
            nc.vector.memset(onesf[:], 1.0)

            # ---- persistent activations ----
            kt_all = ppool.tile([128, TOK], F32R)  # RoPE'd K^T (2 heads stacked)
            # token-major V per 128-token block, per-head [64 V | 1 ones] slots
            v_all = ppool.tile([128, 2 * NKB_B, HPC * (HD + 1)], F32R)

            def rope(dst, src_ps, s0):
                """dst[128,ST] = src*cos + (P_signed @ src)*sin."""
                stg = rpool.tile([128, ST], F32R, tag="stg")
                nc.vector.tensor_copy(stg[:], src_ps[:])
                rot = pspool.tile([128, ST], F32, tag="scr", bufs=3, name="rot")
                nc.tensor.matmul(rot[:], prot_sb[:], stg[:], start=True, stop=True)
                t1 = rpool.tile([128, ST], F32, tag="t1")
                nc.vector.tensor_mul(t1[:], stg[:], cos_sb[:, s0 : s0 + ST])
                t2 = rpool.tile([128, ST], F32, tag="t2")
                nc.vector.tensor_mul(t2[:], rot[:], sin_sb[:, s0 : s0 + ST])
                nc.vector.tensor_add(dst, t1[:], t2[:])

            # ---- per-pair (2-tile) DRAM staging for the AllGather ----
            NPAIR = NST // 2
            ag_in = {}
            ag_out = {}
            for pr in range(NPAIR):
                ag_in[pr] = dpool.tile([128, 2 * ST], BF16, name=f"ag_in{pr}")
                ag_out[pr] = dpool.tile(
                    [D, 2 * ST], BF16, addr_space="Shared", name=f"ag_out{pr}"
                )
            # warmup collective: absorbs the cc-stream cold start + launch skew
            # while tile 0's QKV is still running; output never read.
            wu_in = dpool.tile([128, 8], BF16, name="wu_in")
            wu_out = dpool.tile([D, 8], BF16, addr_space="Shared", name="wu_out")
            nc.gpsimd.collective_compute(
                "AllGather",
                mybir.AluOpType.bypass,
                replica_groups=[list(range(NCORE))],
                ins=[wu_in.opt()],
                outs=[wu_out.opt()],
            )

            q_tiles = {}

            def emit_qkv_tile(st):
                s0 = (st % NQT) * ST  # within-batch position (cos/sin index)
                tok0 = st * ST
                xt_sb = xpool.tile([128, DCH, ST], F32R, tag="xt", name=f"xt{st}")
                xr = xT_d.rearrange("(a p) m -> p a m", p=128)
                if st == 0:  # split so the first matmuls start sooner
                    nc.sync.dma_start(xt_sb[:, 0:4, :], xr[:, 0:4, tok0 : tok0 + ST])
                    nc.sync.dma_start(
                        xt_sb[:, 4:DCH, :], xr[:, 4:DCH, tok0 : tok0 + ST]
                    )
                else:
                    nc.sync.dma_start(xt_sb[:], xr[:, :, tok0 : tok0 + ST])
                qt_sb = rpool.tile([128, ST], F32R, tag="qt", name=f"qt{st}")
                q_tiles[st] = qt_sb
                for part, w_sb in (("q", wq_sb), ("k", wk_sb), ("v", wv_sb)):
                    acc = pspool.tile(
                        [128, ST], F32, tag="qkv", bufs=2, name=f"ps_{part}{st}"
                    )
                    for dk in range(DCH):
                        nc.tensor.matmul(
                            acc[:],
                            w_sb[:, dk, :],
                            xt_sb[:, dk, :],
                            start=(dk == 0),
                            stop=(dk == DCH - 1),
                        )
                    if part == "q":
                        rope(qt_sb[:], acc, s0)
                    elif part == "k":
                        rope(kt_all[:, tok0 : tok0 + ST], acc, s0)
                    else:
                        vs = rpool.tile([128, ST], F32, tag="stg", name=f"vs{st}")
                        nc.vector.tensor_copy(vs[:], acc[:])
                        slot = st * (ST // KB)
                        tr = pspool.tile(
                            [128, ST], F32, tag="scr", bufs=3, name=f"tr{st}"
                        )
                        for jj in range(ST // KB):
                            nc.tensor.transpose(
                                tr[:, jj * 128 : (jj + 1) * 128],
                                vs[:, jj * 128 : (jj + 1) * 128],
                                id_sb[:],
                            )
                        trv = tr[:].rearrange("p (j t h) -> p j t h", j=4, t=HPC)
                        for h in range(HPC):
                            c0 = h * (HD + 1)
                            nc.vector.tensor_copy(
                                v_all[:, slot : slot + 4, c0 : c0 + HD],
                                trv[:, :, h, :],
                            )
                            for jj in range(ST // KB):
                                nc.gpsimd.tensor_copy(
                                    v_all[:, slot + jj, c0 + HD : c0 + HD + 1],
                                    onesf[:],
                                )

            def emit_attention(b, qt):
                st = b * NQT + qt
                q0 = qt * ST  # within this core's qt_sb (always full tile)
                nkb = (qt + 1) * (ST // KB)
                qt_sb = q_tiles.pop(st)
                o_ps = [
                    pspool.tile(
                        [HD + 1, ST], F32, tag="o", bufs=2, name=f"o{b}_{qt}_{h}"
                    )
                    for h in range(HPC)
                ]
                e_tiles = {}

                def emit_scores(kbi):
                    k0 = b * S + kbi * KB
                    for h in range(HPC):
                        stp = pspool.tile(
                            [128, ST],
                            F32,
                            tag="scr",
                            bufs=3,
                            name=f"st{b}_{qt}_{kbi}_{h}",
                        )
                        nc.tensor.matmul(
                            stp[:],
                            kt_all[h * HD : (h + 1) * HD, k0 : k0 + KB],
                            qt_sb[h * HD : (h + 1) * HD, :],
                            start=True,
                            stop=True,
                        )
                        e_sb = epool.tile(
                            [128, ST], F32R, tag="e", name=f"e{b}_{qt}_{kbi}_{h}"
                        )
                        nc.scalar.activation(e_sb[:], stp[:], EXP, scale=SCALE)
                        j = kbi - qt * (ST // KB)
                        if j >= 0:
                            # causal: keep where q - k - j*128 >= 0
                            nc.gpsimd.affine_select(
                                out=e_sb[:],
                                in_=e_sb[:],
                                compare_op=mybir.AluOpType.is_ge,
                                fill=0.0,
                                base=-j * KB,
                                pattern=[[1, ST]],
                                channel_multiplier=-1,
                            )
                        e_tiles[kbi, h] = e_sb

                def emit_pv(kbi):
                    slot = b * NKB_B + kbi
                    for h in range(HPC):
                        c0 = h * (HD + 1)
                        nc.tensor.matmul(
                            o_ps[h][:],
                            v_all[:, slot, c0 : c0 + HD + 1],
                            e_tiles.pop((kbi, h))[:],
                            start=(kbi == 0),
                            stop=(kbi == nkb - 1),
                        )

                emit_scores(0)
                for kbi in range(1, nkb):
                    emit_scores(kbi)
                    emit_pv(kbi - 1)
                emit_pv(nkb - 1)

                # normalize: at = o2 * broadcast(1/z); 1/z = exp(-ln z) on ScalarE
                o2_sb = zpool.tile([128, ST], F32, tag="osb")
                zbc = pspool.tile([128, ST], F32, tag="scr", bufs=3, name=f"zbc{st}")
                nrcp = zpool.tile([64, ST], F32R, tag="nrcp", name=f"nrcp{st}")
                for h in range(HPC):
                    nc.any.tensor_copy(
                        o2_sb[h * HD : (h + 1) * HD, :], o_ps[h][0:HD, :]
                    )
                    lnz = zpool.tile([1, ST], F32, tag="lnz", name=f"lnz{st}_{h}")
                    nc.scalar.activation(lnz[:], o_ps[h][HD : HD + 1, :], LN)
                    # 1/z row parked at partition h*32 so it base-matches zsel
                    nc.scalar.activation(
                        nrcp[h * 32 : h * 32 + 1, :], lnz[:], EXP, scale=-1.0
                    )
                    # accumulate per-head broadcast rows into one [128, ST] tile
                    nc.tensor.matmul(
                        zbc[:],
                        zsel_sb[h * 32 : h * 32 + 1, :],
                        nrcp[h * 32 : h * 32 + 1, :],
                        start=(h == 0),
                        stop=(h == HPC - 1),
                    )
                at_sb = apool.tile([128, ST], BF16, tag="at")
                nc.vector.tensor_mul(at_sb[:], o2_sb[:], zbc[:])
                nc.sync.dma_start(
                    ag_in[st // 2][:, (st % 2) * ST : (st % 2 + 1) * ST], at_sb[:]
                )

            def emit_ag(pr):
                nc.gpsimd.collective_compute(
                    "AllGather",
                    mybir.AluOpType.bypass,
                    replica_groups=[list(range(NCORE))],
                    ins=[ag_in[pr].opt()],
                    outs=[ag_out[pr].opt()],
                )

            def emit_outproj(pr):
                # deferred 2 tiles after the AG trigger so this sync-queue DMA
                # never blocks later xt loads behind an in-flight collective
                ag_sb = gpool.tile(
                    [128, DCH, 2 * ST], BF16, tag="ag", name=f"ag{pr}"
                )
                nc.sync.dma_start(
                    ag_sb[:], ag_out[pr].rearrange("(a p) m -> p a m", p=128)
                )
                for half in range(2):
                    cs = slice(half * ST, (half + 1) * ST)
                    op_ps = pspool.tile(
                        [128, ST], F32, tag="qkv", bufs=2, name=f"op{pr}_{half}"
                    )
                    for fk in range(DCH):
                        nc.tensor.matmul(
                            op_ps[:],
                            wo_sb[:, fk, :],
                            ag_sb[:, fk, cs],
                            start=(fk == 0),
                            stop=(fk == DCH - 1),
                        )
                    yt_sb = ypool.tile([128, ST], F32, tag="yt")
                    nc.vector.tensor_copy(yt_sb[:], op_ps[:])
                    nc.sync.dma_start(
                        yt_d[:, (2 * pr + half) * ST : (2 * pr + half + 1) * ST],
                        yt_sb[:],
                    )

            emit_qkv_tile(0)
            for st in range(NST):
                b, qt = st // NQT, st % NQT
                if st + 1 < NST:
                    emit_qkv_tile(st + 1)
                emit_attention(b, qt)
                if st % 2 == 1:
                    emit_ag(st // 2)
                    if st // 2 >= 1:
                        emit_outproj(st // 2 - 1)
            emit_outproj(NPAIR - 1)

    nc.compile()
    return nc


def _host_tables():
    inv_freq = 1.0 / (ROPE_BASE ** (np.arange(0, HD, 2, dtype=np.float32) / HD))
    t = np.arange(S, dtype=np.float32)
    freqs = np.outer(t, inv_freq)  # [S, 32]
    emb = np.concatenate([freqs, freqs], axis=-1)  # [S, 64]
    cos = np.cos(emb).astype(np.float32)
    sin = np.sin(emb).astype(np.float32)
    cosT2 = np.ascontiguousarray(np.concatenate([cos.T, cos.T], axis=0))  # [128,S]
    sinT2 = np.ascontiguousarray(np.concatenate([sin.T, sin.T], axis=0))
    # signed rotate-half permutation, stacked per 64-row head block, transposed
    # for use as a matmul stationary (rot = prot^T @ x).
    R = np.zeros((128, 128), dtype=np.float32)
    for h0 in (0, 64):
        for r in range(32):
            R[h0 + r, h0 + 32 + r] = -1.0
            R[h0 + 32 + r, h0 + r] = 1.0
    prot = np.ascontiguousarray(R.T)
    # z-broadcast selector: row h*32 -> partitions h*64..(h+1)*64 (rows at
    # 32-multiples because engine partition bases must be multiples of 32)
    zsel = np.zeros((64, 128), dtype=np.float32)
    for h in range(HPC):
        zsel[h * 32, h * HD : (h + 1) * HD] = 1.0
    return cosT2, sinT2, prot, zsel


def _get_nc():
    if "nc" not in _CACHE:
        _CACHE["nc"] = _build_program()
        _CACHE["tables"] = _host_tables()
    return _CACHE["nc"]


def _make_in_maps(x, w_in, w_out):
    cosT2, sinT2, prot, zsel = _CACHE["tables"]
    import ml_dtypes

    xT = np.ascontiguousarray(x.reshape(TOK, D).T)  # [D, TOK]
    in_maps = []
    for c in range(NCORE):
        r = slice(c * 128, (c + 1) * 128)
        in_maps.append(
            {
                "xT": xT,
                "wq": np.ascontiguousarray(w_in[0 * D :][r.start : r.stop].T),
                "wk": np.ascontiguousarray(w_in[1 * D :][r.start : r.stop].T),
                "wv": np.ascontiguousarray(w_in[2 * D :][r.start : r.stop].T),
                "wo": np.ascontiguousarray(
                    w_out[r, :].T.astype(ml_dtypes.bfloat16)
                ),
                "cosT": cosT2,
                "sinT": sinT2,
                "prot": prot,
                "zsel": zsel,
            }
        )
    return in_maps


def kernel(x: np.ndarray, w_in: np.ndarray, w_out: np.ndarray) -> np.ndarray:
    x = np.asarray(x, dtype=np.float32)
    w_in = np.asarray(w_in, dtype=np.float32)
    w_out = np.asarray(w_out, dtype=np.float32)

    nc = _get_nc()
    in_maps = _make_in_maps(x, w_in, w_out)
    _CACHE["last_in_maps"] = in_maps
    res = run_bass_kernel_spmd(nc, in_maps, core_ids=list(range(NCORE)))
    yT = np.concatenate([res.results[c]["yt"] for c in range(NCORE)], axis=0)
    return np.ascontiguousarray(yT.T).reshape(B, S, D)
